# revision 53
# baseline (speedup 1.0000x reference)
"""EnergyTransformerLayer on 8 Trainium2 NeuronCores (Bass/Tile).

Sharding (per spec hint): heads are sharded across the 8 cores (2 heads each)
for the 5-step energy-descent loop; Q_opt is exchanged with an AllToAll before
the Wo projection; the Wo projection + residual + FFN are sharded by target
rows (128 rows per core), so the host assembles the final output by
concatenating per-core row blocks.

Key optimizations over the v1 baseline:
  - The energy loop is jointly scheduled across four engines: PE does the
    score/update matmuls (software-pipelined with a 2-chunk lag so PE never
    waits on exp), ACT computes exp for ~23/32 chunks per step, DVE computes
    exp for the rest via a 2-instruction custom op ((1+t/1024)^1024 squaring
    chain - scores are tightly centered so the approximation error is ~1e-6),
    and the Pool/GPSIMD queue does the q-update elementwise tail.
  - Reciprocal uses the fast approximate custom DVE op.
  - The FFN computes H^T = W1 @ t2^T per 128-hidden-chunk, applies gelu
    straight out of PSUM, and feeds W2 immediately - no G transpose barrier.
  - Wo projection / residual / tanh / transposes are split in halves to
    pipeline, and the A2A staging DMA is issued per t-half as the last
    descent step retires.

Softmax-free descent step (per head, transposed layout):
    scoresT[k, t] = sum_z K[k, z] q[t, z]            (MM1, z=64 row-packed x2)
    ex = exp(beta * scoresT)                         (ACT or DVE, PSUM->SBUF)
    upd = [K | 1/step]^T @ ex                        (MM2: rows 0-63 = num,
                                                      rows 64-127 = rowsum/step)
    qT += num * reciprocal(rowsum/step)              (DVE recip + Pool mul/add)
"""
import numpy as np
import ml_dtypes

import concourse.bass as bass
import concourse.mybir as mybir
import concourse.tile as tile
from concourse import bacc
from concourse.bass_utils import run_bass_kernel_spmd
from concourse.masks import make_identity

dt = mybir.dt
AF = mybir.ActivationFunctionType

N_CORES = 8
EMBED = 1024
N_HEADS = 16
HD = 64
HIDDEN = 4096
N_CTX = 2048
N_TGT = 1024
STEPS = 5
BETA = 1.0 / 8.0          # BETA / sqrt(HD)
INV_STEP = 10.0           # 1 / STEP_SIZE, folded into the ones-block of K_aug

HPC = N_HEADS // N_CORES  # heads per core = 2
TPC = N_TGT // N_CORES    # target rows per core = 128

BF = dt.bfloat16
F32 = dt.float32

# swappable for simulation (CoreSim implements no gelu variant)
GELU_FN = AF.Gelu_apprx_tanh
SPLIT_IN_DMA = True

DC = EMBED // 128     # 8 d-chunks
KC = N_CTX // 128     # 16 k-chunks
HC = HIDDEN // 128    # 32 hidden-chunks

ACT_LAG = 4           # MM2(kc) issued after MM1(kc+lag); bigger lag for the
DVE_LAG = 7           # slower DVE-exp chunks so in-order PE never blocks
# k-chunks whose exp runs on DVE (rest on ACT), per t-half; early-mid
# placement so the end-of-half MM2 drain never waits on the slower DVE exp
DVE_CHUNKS = {0: (3, 6, 9, 12), 1: (2, 5, 8, 11, 14)}
N_WARMUP = 24         # PE keep-warm matmuls spanning the AllToAll window
EXP_N = 1024.0        # exp(t) ~ (1 + t/EXP_N)^EXP_N
W_SCALE = 256.0       # host-side fp8 scale for W1/W2 (undone on device)
F8 = dt.float8e4

# ---------------------------------------------------------------------------
# Custom DVE ops: exp via squaring chain, 2 instructions.
#   A: v = (1 + x*C0)^32      (C0 = beta/EXP_N)
#   B: out = v^32             (=> (1+x*C0)^1024)
# ---------------------------------------------------------------------------


def _register_exp_ops():
    from concourse.dve_spec import Spec, Src0, One, C0, lower
    from concourse.dve_ops import (
        DveOp, OPS, CUSTOM_DVE_SPECS, _SUB_OPCODE_FOR_NAME,
        _CUSTOM_DVE_ROW_BASE, has_src1,
    )
    from concourse.dve_uop import DveOpSpec
    from concourse.dve_table_gen import dve_ver_for

    if "EXP_SQCHAIN_A_ANT" in _SUB_OPCODE_FOR_NAME:
        from concourse import dve_ops
        return (dve_ops.EXP_SQCHAIN_A_ANT, dve_ops.EXP_SQCHAIN_B_ANT,
                dve_ops.DIV_APPROX_ANT)

    from concourse.dve_spec import Src1, Bin, AluOp as SAluOp, C1

    u = Src0 * C0 + One
    for _ in range(5):
        u = u * u

    def ref_a(in0, in1, c0, c1, c2):
        v = (np.float32(1.0)
             + in0.astype(np.float32) * np.asarray(c0, np.float32))
        v = v.astype(np.float32)
        for _ in range(5):
            v = (v * v).astype(np.float32)
        return v

    w = Src0 * Src0
    for _ in range(4):
        w = w * w

    def ref_b(in0, in1, c0, c1, c2):
        v = in0.astype(np.float32)
        for _ in range(5):
            v = (v * v).astype(np.float32)
        return v

    # out = Src0 / Src1 (approx): BITWISE_NOT exponent-flip seed + one
    # inline Newton pass (~0.4% rel err - only used on the softmax
    # denominator). Same Chebyshev pair as RECIPROCAL_APPROX_FAST.
    _n = Bin(SAluOp.BITWISE_NOT, Src1, Src1)
    y0 = _n * C0
    y1 = y0 * (C1 - Src1 * y0)

    def ref_div(in0, in1, c0, c1, c2):
        nx = (~np.asarray(in1, np.float32).view(np.int32)).view(np.float32)
        y0r = nx * np.float32(c0 if np.isscalar(c0) else np.asarray(c0))
        y1r = y0r * (np.float32(np.asarray(c1)) - in1 * y0r)
        return (in0 * y1r).astype(np.float32)

    ops = []
    for name, spec in [
        ("EXP_SQCHAIN_A_ANT", Spec(body=u, reference=ref_a)),
        ("EXP_SQCHAIN_B_ANT", Spec(body=w, reference=ref_b)),
        ("DIV_APPROX_ANT", Spec(body=Src0 * y1, reference=ref_div)),
    ]:
        row = _CUSTOM_DVE_ROW_BASE + len(OPS)
        assert row < 0x20, "custom-DVE opcode rows exhausted"
        op = DveOp(name, spec, subdim=False, uops_sha={})
        # pin the sha self-consistently (computed from this process's lower())
        for ver in ("v3", "v4"):
            try:
                lowered = DveOpSpec(
                    name=name, opcode=row, uops=lower(spec, ver=ver),
                    rd1_en=has_src1(spec),
                )
                op.uops_sha[ver] = lowered.sha(ver)
            except Exception:
                pass
        OPS.append(op)
        CUSTOM_DVE_SPECS[name] = spec
        _SUB_OPCODE_FOR_NAME[name] = row
        import concourse.dve_ops as dve_ops_mod
        setattr(dve_ops_mod, name, op)
        ops.append(op)
    return ops


EXP_A, EXP_B, DIV_APPROX = _register_exp_ops()


def build_kernel(replicas: int = 1, no_collective: bool = False,
                 loop_n: int = 1, gate_weights: bool = True,
                 skip_tail: bool = False):
    """Build the SPMD Bacc program (same NEFF on all 8 cores).

    no_collective=True replaces the AllToAll with a local DRAM copy - only
    for timing/timeline analysis. loop_n>1 wraps the body in a hardware
    For_i loop for precise slope timing.
    """
    nc = bacc.Bacc("TRN2", target_bir_lowering=False, debug=False,
                   num_devices=N_CORES)

    ctxT_d = nc.dram_tensor("ctxT", [EMBED, N_CTX], BF, kind="ExternalInput")
    tgtT_d = nc.dram_tensor("tgtT", [EMBED, N_TGT], BF, kind="ExternalInput")
    tgt_rows_d = nc.dram_tensor("tgt_rows", [TPC, EMBED], F32, kind="ExternalInput")
    wqkT_d = nc.dram_tensor("wqkT", [EMBED, 2 * HPC * HD], BF, kind="ExternalInput")
    woT_d = nc.dram_tensor("woT", [EMBED, EMBED], BF, kind="ExternalInput")
    w1T_d = nc.dram_tensor("w1T", [EMBED, HIDDEN], BF, kind="ExternalInput")
    w2T_d = nc.dram_tensor("w2T", [HIDDEN, EMBED], BF, kind="ExternalInput")
    alphas_d = nc.dram_tensor("alphas", [128, 2], F32, kind="ExternalInput")
    out_d = nc.dram_tensor("out_rows", [TPC, EMBED], F32, kind="ExternalOutput")

    with tile.TileContext(nc) as tc:
        with (
            tc.tile_pool(name="const", bufs=1) as cpool,
            tc.tile_pool(name="persist", bufs=1) as pp,
            tc.tile_pool(name="wts", bufs=1) as wp,
            tc.tile_pool(name="stream", bufs=2) as sp,
            tc.tile_pool(name="work", bufs=1) as wk,
            tc.tile_pool(name="psA", bufs=3, space="PSUM") as psA,  # [128,1024]f32: 2 banks
            tc.tile_pool(name="psB", bufs=2, space="PSUM") as psB,  # [128,512]f32: 1 bank
            tc.tile_pool(name="dram", bufs=1, space="DRAM") as dp,
        ):
            alphas = cpool.tile([128, 2], F32)
            nc.sync.dma_start(out=alphas[:], in_=alphas_d[:])
            ident = cpool.tile([128, 128], BF)
            make_identity(nc, ident[:])

            wqkT = cpool.tile([128, DC * 256], BF)        # [d-chunk | wq128 wk128]
            nc.sync.dma_start(
                out=wqkT[:].rearrange("p (a f) -> p a f", a=DC),
                in_=wqkT_d.rearrange("(a p) f -> p a f", p=128),
            )
            woT_sb = wp.tile([128, DC * EMBED], BF)       # [d-chunk | e]

            def body(rep):
                # ------------- phase 1+2: tnorm, K / q projections ----------
                KT = pp.tile([128, N_CTX], BF, tag="KT", name=f"KT{rep}")
                Kaug = pp.tile([128, KC * 2 * 128], BF, tag="Kaug",
                               name=f"Kaug{rep}")
                nc.vector.memset(Kaug[:], INV_STEP)
                qT = pp.tile([128, N_TGT], F32, tag="qT", name=f"qT{rep}")

                # dependency-free warmups ramp the PE clock while the first
                # ctx/tgt chunks stream in, so the projection matmuls start
                # at full speed instead of the low p-state.
                wuh = psB.tile([128, 128], F32, tag="psB", name=f"wuh{rep}")
                for i in range(16):
                    nc.tensor.matmul(wuh[:], ident[:], ident[:],
                                     start=True, stop=True,
                                     skip_group_check=True)

                kps = [psA.tile([128, 1024], F32, tag="psA", name=f"kps{rep}_{i}")
                       for i in range(2)]
                qps = psA.tile([128, 1024], F32, tag="psA", name=f"qps{rep}")
                last_in_dma = None
                nsp = 2 if SPLIT_IN_DMA else 1
                for d in range(DC):
                    ctx_t = sp.tile([128, N_CTX], BF, tag="ctx", name=f"ctx{rep}_{d}")
                    cw = N_CTX // nsp
                    for hh in range(nsp):
                        last_in_dma = nc.sync.dma_start(
                            out=ctx_t[:, hh * cw:(hh + 1) * cw],
                            in_=ctxT_d.rearrange("(a p) k -> p a k", p=128)[
                                :, d, hh * cw:(hh + 1) * cw],
                        )
                    # tgt stream dispatches from the ACT hwdge queue so the
                    # SP sequencer (565ns/DMA) isn't the ramp bottleneck
                    tgt_t = sp.tile([128, N_TGT], BF, tag="tgt", name=f"tgt{rep}_{d}")
                    tw = N_TGT // nsp
                    for hh in range(nsp):
                        nc.scalar.dma_start(
                            out=tgt_t[:, hh * tw:(hh + 1) * tw],
                            in_=tgtT_d.rearrange("(a p) t -> p a t", p=128)[
                                :, d, hh * tw:(hh + 1) * tw],
                        )
                    tn_t = sp.tile([128, N_TGT], BF, tag="tn", bufs=1,
                                   name=f"tn{rep}_{d}")
                    nc.scalar.activation(tn_t[:], tgt_t[:], AF.Tanh,
                                         scale=alphas[:, 0:1])
                    wq = wqkT[:, d * 256:d * 256 + 128]
                    wkk = wqkT[:, d * 256 + 128:d * 256 + 256]
                    first, last = d == 0, d == DC - 1
                    for kcol in range(4):
                        nc.tensor.matmul(
                            kps[kcol // 2][:, (kcol % 2) * 512:(kcol % 2 + 1) * 512],
                            wkk, ctx_t[:, kcol * 512:(kcol + 1) * 512],
                            start=first, stop=last)
                    for tcol in range(2):
                        nc.tensor.matmul(
                            qps[:, tcol * 512:(tcol + 1) * 512],
                            wq, tn_t[:, tcol * 512:(tcol + 1) * 512],
                            start=first, stop=last)
                    # keep the PE clock ramped through the DMA-paced gaps
                    # between projection chunks
                    for i in range(3):
                        nc.tensor.matmul(wuh[:], ident[:], ident[:],
                                         start=True, stop=True,
                                         skip_group_check=True)
                for i in range(2):
                    nc.vector.tensor_copy(
                        KT[:, i * 1024:(i + 1) * 1024], kps[i][:])
                nc.vector.tensor_copy(qT[:], qps[:])

                # transpose K_hT -> K_aug blocks ([k, z] layout per head)
                for kc in range(KC):
                    ktp = psB.tile([128, 128], BF, tag="psB", name=f"ktp{rep}_{kc}")
                    nc.tensor.transpose(ktp[:], KT[:, kc * 128:(kc + 1) * 128],
                                        ident[:])
                    base = kc * 256
                    nc.vector.tensor_copy(
                        Kaug[:, base:base + 256].rearrange(
                            "p (h f) -> p h f", f=128)[:, :, 0:64],
                        ktp[:].rearrange("p (h f) -> p h f", f=64),
                    )

                # FFN / Wo weight streaming: emit DMAs early so the queues
                # stay busy during the descent loop, gated behind the ramp.
                from concourse.tile import add_dep_helper

                gate = last_in_dma.ins
                w1cs, w2cs = [], []
                for a in range(DC):
                    wd = nc.sync.dma_start(
                        out=woT_sb[:, a * EMBED:(a + 1) * EMBED],
                        in_=woT_d.rearrange("(a p) e -> p a e", p=128)[:, a, :],
                    )
                    if gate_weights:
                        add_dep_helper(wd.ins, gate, sync=True,
                                       reason="after ramp")
                for q in range(4):
                    w1c = wp.tile([128, DC * 1024], BF, tag="w1s", bufs=4,
                                  name=f"w1c{rep}_{q}")
                    for a in range(DC):
                        wd = nc.sync.dma_start(
                            out=w1c[:, a * 1024:(a + 1) * 1024],
                            in_=w1T_d.rearrange("(a p) h -> p a h", p=128)[
                                :, a, q * 1024:(q + 1) * 1024],
                        )
                        if gate_weights:
                            add_dep_helper(wd.ins, gate, sync=True,
                                           reason="after ramp")
                    w1cs.append(w1c)
                def load_w2(q, gated):
                    w2c = wp.tile([128, 8 * EMBED], BF, tag="w2s", bufs=2,
                                  name=f"w2c{rep}_{q}")
                    for j in range(8):
                        hc = q * 8 + j
                        wd = nc.sync.dma_start(
                            out=w2c[:, j * EMBED:(j + 1) * EMBED],
                            in_=w2T_d.rearrange("(a p) e -> p a e", p=128)[:, hc, :],
                        )
                        if gated and gate_weights:
                            add_dep_helper(wd.ins, gate, sync=True,
                                           reason="after ramp")
                    w2cs.append(w2c)

                for q in range(2):
                    load_w2(q, gated=True)
                # residual rows for phase 5: load during the loop
                tgt_r = wk.tile([128, EMBED], F32, tag="tgt_r", name=f"tgtr{rep}")
                trd = nc.sync.dma_start(out=tgt_r[:], in_=tgt_rows_d[:])
                if gate_weights:
                    add_dep_helper(trd.ins, gate, sync=True, reason="after ramp")

                # ------------- phase 3: 5-step energy descent ---------------
                qbf = {}
                for th in range(2):
                    tsl = slice(th * 512, (th + 1) * 512)
                    b = wk.tile([128, 512], BF, tag=f"qbf{th}", bufs=1,
                                name=f"qbf{rep}_init{th}")
                    nc.gpsimd.tensor_copy(b[:], qT[:, tsl])
                    qbf[th] = b

                # A2A staging buffers (filled per t-half as step 5 retires)
                qfin = wk.tile([128, N_TGT], BF, tag="qfin", name=f"qfin{rep}")
                q_loc = dp.tile([N_CORES * 128, TPC], BF, name=f"qloc{rep}")
                q_ex = dp.tile([N_CORES * 128, TPC], BF, name=f"qex{rep}")

                for step in range(STEPS):
                    last_step = step == STEPS - 1
                    for th in range(2):
                        tsl = slice(th * 512, (th + 1) * 512)
                        dve_set = DVE_CHUNKS[th]
                        upd = [psB.tile([128, 512], F32, tag="psB",
                                        name=f"upd{rep}_{step}_{th}_{h}")
                               for h in range(2)]
                        exs = {}

                        def mm1_exp(kc2):
                            sc = psA.tile([128, 1024], F32, tag="psA",
                                          name=f"sc{rep}_{step}_{th}_{kc2}")
                            for h in range(2):
                                nc.tensor.matmul(
                                    sc[:, h * 512:(h + 1) * 512],
                                    KT[h * 64:(h + 1) * 64,
                                       kc2 * 128:(kc2 + 1) * 128],
                                    qbf[th][h * 64:(h + 1) * 64, :],
                                    start=True, stop=True,
                                )
                            ex = wk.tile([128, 1024], BF, tag="ex", bufs=7,
                                         name=f"ex{rep}_{step}_{th}_{kc2}")
                            if kc2 in dve_set:
                                vt = wk.tile([128, 1024], F32, tag="vexp",
                                             bufs=1,
                                             name=f"vx{rep}_{step}_{th}_{kc2}")
                                nc.vector._custom_dve(
                                    EXP_A, out=vt[:], in0=sc[:],
                                    s0=BETA / EXP_N)
                                nc.vector._custom_dve(
                                    EXP_B, out=ex[:], in0=vt[:])
                            else:
                                nc.scalar.activation(ex[:], sc[:], AF.Exp,
                                                     scale=BETA)
                            exs[kc2] = ex

                        # mm2 emission schedule: chunk kc retires at slot
                        # kc+lag (bigger lag for DVE chunks); start/stop
                        # flags follow emission order.
                        sched = {}
                        mm2_order = []
                        for kc in range(KC):
                            lag = DVE_LAG if kc in dve_set else ACT_LAG
                            sched.setdefault(kc + lag, []).append(kc)
                        for slot in sorted(sched):
                            mm2_order.extend(sched[slot])

                        def mm2(kc2):
                            ex = exs.pop(kc2)
                            for h in range(2):
                                nc.tensor.matmul(
                                    upd[h][:],
                                    Kaug[:, kc2 * 256 + h * 128:
                                         kc2 * 256 + (h + 1) * 128],
                                    ex[:, h * 512:(h + 1) * 512],
                                    start=(kc2 == mm2_order[0]),
                                    stop=(kc2 == mm2_order[-1]),
                                )

                        for kc in range(KC):
                            mm1_exp(kc)
                            for kc2 in sched.pop(kc, []):
                                mm2(kc2)
                        for slot in sorted(sched):
                            for kc2 in sched[slot]:
                                mm2(kc2)

                        # tail: qT[:, tsl] += num / den. recip + mult per head
                        # on DVE (only one PSUM operand allowed per
                        # instruction); the mult frees the upd PSUM tile fast
                        # enough that the next t-half's MM2s never block.
                        # tail: ACT stages den in SBUF (only one PSUM input
                        # allowed per instruction), then one fused DVE op per
                        # head computes dq = num * recip_approx(den); all
                        # custom-op APs start at partition 0 (HW requirement).
                        from concourse.dve_ops import RECIP_APPROX_FAST_CONSTS
                        _rc = RECIP_APPROX_FAST_CONSTS
                        dqs = []
                        for h in range(2):
                            den = wk.tile([64, 512], F32, tag=f"den{h}",
                                          bufs=1, name=f"den{rep}_{step}_{th}_{h}")
                            nc.scalar.activation(den[:], upd[h][64:128, :],
                                                 AF.Copy)
                            dq = wk.tile([64, 512], F32, tag=f"dq{h}", bufs=1,
                                         name=f"dq{rep}_{step}_{th}_{h}")
                            nc.vector._custom_dve(
                                DIV_APPROX, out=dq[:], in0=upd[h][0:64, :],
                                in1=den[:], s0=_rc["s0"], s1=_rc["s1"],
                            )
                            dqs.append(dq)
                        # h1's dq is staged to partition 64 first: a
                        # TensorTensor with both inputs in SBUF requires
                        # equal base partitions on HW.
                        dqc = wk.tile([128, 512], F32, tag="dqc", bufs=1,
                                      name=f"dqc{rep}_{step}_{th}")
                        nc.gpsimd.tensor_copy(dqc[64:128, :], dqs[1][:])
                        dq_in = {0: dqs[0][:], 1: dqc[64:128, :]}

                        if not last_step:
                            for h in range(2):
                                hsl = slice(h * 64, (h + 1) * 64)
                                nc.gpsimd.tensor_tensor(
                                    qT[hsl, tsl], qT[hsl, tsl], dq_in[h],
                                    mybir.AluOpType.add,
                                )
                            b = wk.tile([128, 512], BF, tag=f"qbf{th}", bufs=1,
                                        name=f"qbf{rep}_{step}_{th}")
                            nc.gpsimd.tensor_copy(b[:], qT[:, tsl])
                            qbf[th] = b
                        elif skip_tail:
                            for h in range(2):
                                hsl = slice(h * 64, (h + 1) * 64)
                                nc.gpsimd.tensor_tensor(
                                    qT[hsl, tsl], qT[hsl, tsl], dq_in[h],
                                    mybir.AluOpType.add,
                                )
                        else:
                            # final update straight to bf16 + stage this
                            # t-half for the A2A immediately
                            for h in range(2):
                                hsl = slice(h * 64, (h + 1) * 64)
                                nc.gpsimd.tensor_tensor(
                                    qfin[hsl, tsl], qT[hsl, tsl], dq_in[h],
                                    mybir.AluOpType.add,
                                )
                            nc.sync.dma_start(
                                out=q_loc[:].rearrange(
                                    "(j p) t -> p j t", p=128)[
                                        :, th * 4:(th + 1) * 4, :],
                                in_=qfin[:, tsl].rearrange(
                                    "p (j t) -> p j t", j=4),
                            )

                if skip_tail:
                    out_sb0 = wk.tile([128, EMBED], F32, tag="out_sb",
                                      name=f"outq{rep}")
                    nc.vector.tensor_copy(out_sb0[:], qT[:])
                    nc.sync.dma_start(out=out_d[:], in_=out_sb0[:])
                    return

                # ------------- phase 4: AllToAll on Q -----------------------
                # q_loc [8*128, TPC]: partition-block j holds my heads' q at
                # t-block j; after A2A, block j holds core j's heads at MY
                # t-block. bf16 halves the collective bytes.
                if no_collective:
                    nc.sync.dma_start(out=q_ex[:], in_=q_loc[:])
                else:
                    nc.gpsimd.collective_compute(
                        "AllToAll",
                        mybir.AluOpType.bypass,
                        replica_groups=[list(range(N_CORES))],
                        ins=[q_loc[:]],
                        outs=[q_ex[:]],
                    )
                # PE keep-warm: dependency-free matmuls bridge the AllToAll
                # window so the tensor engine's clock doesn't drop to the
                # low p-state right before the Wo projection.
                wu = psA.tile([128, 1024], F32, tag="psA", name=f"wu{rep}")
                for i in range(N_WARMUP):
                    nc.tensor.matmul(
                        wu[:, (i % 2) * 512:(i % 2 + 1) * 512],
                        ident[:],
                        woT_sb[:, (i % 16) * 512:(i % 16 + 1) * 512],
                        start=True, stop=True, skip_group_check=True,
                    )
                qto = wk.tile([128, DC * TPC], BF, tag="qto", name=f"qto{rep}")
                nc.sync.dma_start(
                    out=qto[:].rearrange("p (a t) -> p a t", a=DC),
                    in_=q_ex[:].rearrange("(a p) t -> p a t", p=128),
                )

                # ------------- phase 5: Wo projection + residual ------------
                atn = psA.tile([128, 1024], F32, tag="psA", name=f"atn{rep}")
                for a in range(DC):
                    for ecol in range(2):
                        nc.tensor.matmul(
                            atn[:, ecol * 512:(ecol + 1) * 512],
                            qto[:, a * TPC:(a + 1) * TPC],
                            woT_sb[:, a * EMBED + ecol * 512:
                                   a * EMBED + (ecol + 1) * 512],
                            start=(a == 0), stop=(a == DC - 1),
                        )
                t2 = pp.tile([128, EMBED], F32, tag="t2", name=f"t2{rep}")
                t2n = wk.tile([128, EMBED], BF, tag="t2n", name=f"t2n{rep}")
                t2T = wk.tile([128, DC * TPC], BF, tag="t2T", name=f"t2T{rep}")
                # halves: add -> tanh -> 4 transposes into one PSUM scratch;
                # a single wide Pool copy then fills t2T (no psB ping-pong).
                scr = psA.tile([128, 1024], BF, tag="psA", name=f"scr{rep}")
                for half in range(2):
                    hsl = slice(half * 512, (half + 1) * 512)
                    nc.vector.tensor_tensor(t2[:, hsl], tgt_r[:, hsl],
                                            atn[:, hsl], mybir.AluOpType.add)
                    nc.scalar.activation(t2n[:, hsl], t2[:, hsl], AF.Tanh,
                                         scale=alphas[:, 1:2])
                    for dd in range(4):
                        d = half * 4 + dd
                        nc.tensor.transpose(scr[:, d * 128:(d + 1) * 128],
                                            t2n[:, d * 128:(d + 1) * 128],
                                            ident[:])
                    nc.vector.tensor_copy(t2T[:, half * 512:(half + 1) * 512],
                                          scr[:, hsl])

                # ------------- phase 6: FFN ---------------------------------
                # per hidden-eighth: H_e = t2 @ W1_e^T (512-wide matmuls),
                # gelu -> G_e, PE-transpose into a shared PSUM scratch, one
                # Pool copy into GT, W2 accumulation (512-wide). Eighth e+2's
                # H overlaps eighth e's W2 pass.
                load_w2(2, gated=False)
                load_w2(3, gated=False)
                GT = wk.tile([128, HIDDEN], BF, tag="GT", name=f"GT{rep}")
                fps = psA.tile([128, 1024], F32, tag="psA", name=f"fps{rep}")
                gscr = [psA.tile([128, 1024], BF, tag="psA",
                                 name=f"gscr{rep}_{i}") for i in range(2)]

                gbufs = {}

                def ffn_h(e):  # one 512-wide eighth of hidden: H + gelu
                    q, j = e // 2, e % 2
                    hp = psB.tile([128, 512], F32, tag="psB",
                                  name=f"hps{rep}_{e}")
                    w1c = w1cs[q]
                    for a in range(DC):
                        nc.tensor.matmul(
                            hp[:],
                            t2T[:, a * TPC:(a + 1) * TPC],
                            w1c[:, a * 1024 + j * 512:a * 1024 + (j + 1) * 512],
                            start=(a == 0), stop=(a == DC - 1),
                        )
                    g = wk.tile([128, 512], BF, tag="G", bufs=3,
                                name=f"G{rep}_{e}")
                    nc.scalar.activation(g[:], hp[:], GELU_FN)
                    gbufs[e] = g

                def ffn_t(e):  # transposes for eighth e -> GT
                    g = gbufs.pop(e)
                    sg = gscr[(e // 2) % 2][:, (e % 2) * 512:(e % 2 + 1) * 512]
                    for c in range(4):
                        nc.tensor.transpose(sg[:, c * 128:(c + 1) * 128],
                                            g[:, c * 128:(c + 1) * 128],
                                            ident[:])
                    nc.vector.tensor_copy(GT[:, e * 512:(e + 1) * 512], sg[:])

                def ffn_o(e):
                    for c in range(4):
                        hc = e * 4 + c
                        w2c = w2cs[hc // 8]
                        for ecol in range(2):
                            nc.tensor.matmul(
                                fps[:, ecol * 512:(ecol + 1) * 512],
                                GT[:, hc * 128:(hc + 1) * 128],
                                w2c[:, (hc % 8) * EMBED + ecol * 512:
                                    (hc % 8) * EMBED + (ecol + 1) * 512],
                                start=(hc == 0), stop=(hc == HC - 1),
                            )

                # PE order: h(0), h(1), T(0), h(2), T(1), o(0), h(3), T(2),
                # o(1), ... - each transpose batch runs right after the next
                # eighth's matmuls so gelu/Pool-copy latency never stalls PE.
                ffn_h(0)
                ffn_h(1)
                ffn_t(0)
                for e in range(8):
                    if e + 2 < 8:
                        ffn_h(e + 2)
                    if e + 1 < 8:
                        ffn_t(e + 1)
                    ffn_o(e)
                out_sb = wk.tile([128, EMBED], F32, tag="out_sb", name=f"out{rep}")
                nc.vector.tensor_tensor(out_sb[:], t2[:], fps[:],
                                        mybir.AluOpType.add)
                nc.sync.dma_start(out=out_d[:], in_=out_sb[:])

            if loop_n > 1:
                assert no_collective and replicas == 1
                with tc.For_i(0, loop_n, 1):
                    body(0)
            else:
                for rep in range(replicas):
                    body(rep)

    nc.compile()
    return nc


def prepare_inputs(context, target, Wq, Wk, Wo, W1, W2, alpha1, alpha2):
    """Per-core host-side layout prep. Returns list of 8 in_maps."""
    bf = ml_dtypes.bfloat16
    context = np.asarray(context, np.float32)
    target = np.asarray(target, np.float32)
    ctxT = np.ascontiguousarray(context.T).astype(bf)            # [1024, 2048]
    tgtT = np.ascontiguousarray(target.T).astype(np.float32)     # [1024, 1024]
    f8 = mybir.dt.np(F8)
    woT = np.ascontiguousarray(np.asarray(Wo, np.float32).T).astype(bf)
    w1T = np.ascontiguousarray(np.asarray(W1, np.float32).T).astype(bf)
    w2T = np.ascontiguousarray(np.asarray(W2, np.float32).T).astype(bf)
    alphas = np.zeros((128, 2), np.float32)
    alphas[:, 0] = np.float32(np.asarray(alpha1).reshape(-1)[0])
    alphas[:, 1] = np.float32(np.asarray(alpha2).reshape(-1)[0])
    Wq = np.asarray(Wq, np.float32)
    Wk = np.asarray(Wk, np.float32)

    tgtT = tgtT.astype(bf)
    in_maps = []
    for c in range(N_CORES):
        hs = slice(c * HPC, (c + 1) * HPC)
        wq = Wq[hs].reshape(HPC * HD, EMBED)
        wkk = Wk[hs].reshape(HPC * HD, EMBED)
        wqkT = np.concatenate(
            [np.ascontiguousarray(wq.T), np.ascontiguousarray(wkk.T)], axis=1
        ).astype(bf)                                             # [1024, 256]
        in_maps.append({
            "ctxT": ctxT,
            "tgtT": tgtT,
            "tgt_rows": np.ascontiguousarray(
                target[c * TPC:(c + 1) * TPC]).astype(np.float32),
            "wqkT": wqkT,
            "woT": woT,
            "w1T": w1T,
            "w2T": w2T,
            "alphas": alphas,
        })
    return in_maps


def kernel(context, target, Wq, Wk, Wo, W1, W2, alpha1, alpha2):
    in_maps = prepare_inputs(context, target, Wq, Wk, Wo, W1, W2,
                             alpha1, alpha2)
    nc = build_kernel()
    res = run_bass_kernel_spmd(nc, in_maps, list(range(N_CORES)))
    out = np.concatenate(
        [res.results[c]["out_rows"] for c in range(N_CORES)], axis=0
    )
    return out.astype(np.float32)
